# revision 1
# baseline (speedup 1.0000x reference)
"""CaptionNet Trainium2 kernel (8-core SPMD, data-parallel over batch).

Per core (batch shard Bc=32): attention-LSTM recurrence fully on-chip in a
feature-on-partition / batch-on-free layout, bf16 matmul operands with fp32
PSUM accumulation. Softmax runs without max-subtraction (logits ~N(0,0.6));
the unnormalized exp(z) column per sample is the stationary operand of the
attention einsum (enc pre-transposed to [B,F,C] on the host, streamed as the
moving operand), run as 3-way column-tiled concurrent matmuls
(tile_position=(0,{0,32,64})); 1/sum(exp) is applied per sample via the ACT
copy `scale` AP. The vocab projection is deferred and batched over all T*Bc
tokens: h-chunks stationary, vocab_W.T streamed from HBM in N-chunks, logits
written out as contiguous [T*Bc, V] rows. The host shards the batch, shifts
the teacher-forced inputs, pre-transposes and pre-casts everything.
"""

import numpy as np
import ml_dtypes

import concourse.bass as bass
import concourse.tile as tile
import concourse.mybir as mybir

BF16 = mybir.dt.bfloat16
F32 = mybir.dt.float32
AF = mybir.ActivationFunctionType
OP = mybir.AluOpType

# Problem constants (full size)
B_FULL, T_FULL, H, WV, F, C, V_FULL = 256, 20, 512, 301, 196, 512, 9871
N_CORES = 8
F_HI = 128
F_LO = F - F_HI  # 68


def _tiles(total, step=128):
    return [(i, min(step, total - i)) for i in range(0, total, step)]


def build_program(Bc=32, T=20, V=V_FULL, vchunk=512, stage=99):
    TB = Bc * T
    nc = bass.Bass()

    # ---------------- DRAM I/O (per-core) ----------------
    encT_d = nc.dram_tensor("encT", [Bc, F, C], BF16, kind="ExternalInput")
    xT_d = nc.dram_tensor("xT", [WV, TB], BF16, kind="ExternalInput")
    AxT_d = nc.dram_tensor("AxT", [WV, F], BF16, kind="ExternalInput")
    AhT_d = nc.dram_tensor("AhT", [H, F], BF16, kind="ExternalInput")
    WxT_d = nc.dram_tensor("WxT", [WV, WV], BF16, kind="ExternalInput")
    WcT_d = nc.dram_tensor("WcT", [C, WV], BF16, kind="ExternalInput")
    gateTa_d = nc.dram_tensor("gateTa", [H + 1, C], BF16, kind="ExternalInput")
    WihT_d = nc.dram_tensor("WihT", [WV, 4 * H], BF16, kind="ExternalInput")
    WhhT_d = nc.dram_tensor("WhhT", [H, 4 * H], BF16, kind="ExternalInput")
    vWT_d = nc.dram_tensor("vWT", [H, V], BF16, kind="ExternalInput")
    attnb_d = nc.dram_tensor("attn_br", [1, F], BF16, kind="ExternalInput")
    combb_d = nc.dram_tensor("comb_br", [1, WV], BF16, kind="ExternalInput")
    lstmb_d = nc.dram_tensor("lstm_bc", [128, 16, Bc], F32, kind="ExternalInput")
    eye_d = nc.dram_tensor("eye", [Bc, Bc], F32, kind="ExternalInput")
    onesc_d = nc.dram_tensor("ones_col", [F, 1], BF16, kind="ExternalInput")
    onesr_d = nc.dram_tensor("ones_row", [1, TB], BF16, kind="ExternalInput")
    out_d = nc.dram_tensor("out", [TB, V], F32, kind="ExternalOutput")

    wv_t = _tiles(WV)   # [(0,128),(128,128),(256,45)]
    h_t = _tiles(H)     # 4 x 128
    f_t = [(0, F_HI), (F_HI, F_LO)]
    NWV, NH, NF = len(wv_t), len(h_t), len(f_t)
    n_mv = _tiles(TB)   # vocab stationary chunks along T*Bc

    with tile.TileContext(nc) as tc:
        with (
            tc.tile_pool(name="w", bufs=1) as wp,
            tc.tile_pool(name="act", bufs=2) as ap,
            tc.tile_pool(name="st", bufs=2) as st,
            tc.tile_pool(name="vo", bufs=3) as vp,
            tc.tile_pool(name="ps1", bufs=1, space="PSUM") as ps1,
            tc.tile_pool(name="ps2", bufs=2, space="PSUM") as ps2,
        ):
            # ---------------- resident loads ----------------
            def load_ktiles(dram, ktiles, ncols, dt, name):
                out = []
                for ki, (k0, ks) in enumerate(ktiles):
                    tl = wp.tile([ks, ncols], dt, tag=f"{name}{ki}", name=f"{name}{ki}")
                    nc.sync.dma_start(tl[:], dram[k0 : k0 + ks, :])
                    out.append(tl)
                return out

            ones_f = load_ktiles(onesc_d, f_t, 1, BF16, "ones")
            ones1 = wp.tile([1, TB], BF16, tag="onesr", name="onesr")
            nc.sync.dma_start(ones1[:], onesr_d[:])
            eye_sb = wp.tile([Bc, Bc], F32, tag="eye", name="eye")
            nc.sync.dma_start(eye_sb[:], eye_d[:])

            enc_sb = []
            encT_r = encT_d.rearrange("b f c -> f b c")
            for fi, (f0, fs) in enumerate(f_t):
                e = wp.tile([fs, Bc, C], BF16, tag=f"enc{fi}", name=f"enc{fi}")
                nc.sync.dma_start(e[:], encT_r[f0 : f0 + fs])
                enc_sb.append(e)

            xT_sb = load_ktiles(xT_d, wv_t, TB, BF16, "xT")
            AxT_sb = load_ktiles(AxT_d, wv_t, F, BF16, "AxT")
            AhT_sb = load_ktiles(AhT_d, h_t, F, BF16, "AhT")
            WxT_sb = load_ktiles(WxT_d, wv_t, WV, BF16, "WxT")
            WcT_sb = load_ktiles(WcT_d, h_t, WV, BF16, "WcT")
            gateT_sb = load_ktiles(gateTa_d, h_t, C, BF16, "gateT")
            gateB_sb = wp.tile([1, C], BF16, tag="gateB", name="gateB")
            nc.sync.dma_start(gateB_sb[:], gateTa_d[H : H + 1, :])
            WihT_sb = load_ktiles(WihT_d, wv_t, 4 * H, BF16, "WihT")
            WhhT_sb = load_ktiles(WhhT_d, h_t, 4 * H, BF16, "WhhT")
            attnb_sb = wp.tile([1, F], BF16, tag="attnbr", name="attnbr")
            nc.sync.dma_start(attnb_sb[:], attnb_d[:])
            combb_sb = wp.tile([1, WV], BF16, tag="combbr", name="combbr")
            nc.sync.dma_start(combb_sb[:], combb_d[:])
            lstmb_sb = wp.tile([128, 16, Bc], F32, tag="lstmb", name="lstmb")
            nc.sync.dma_start(lstmb_sb[:], lstmb_d[:])

            h_all = wp.tile([128, NH, TB], BF16, tag="h_all", name="h_all")

            if stage < 1:
                return nc
            # ---------------- x-precomputes: zx = x@Ax.T ; cx = x@Wx.T ----------------
            NN = min(320, TB)

            def precompute(weights, mtiles, dst_tiles, bias_row):
                for mi, (m0, ms) in enumerate(mtiles):
                    for n0 in range(0, TB, NN):
                        nn = min(NN, TB - n0)
                        pps = ps1.tile([128, NN], F32, tag="sm", name="sm")
                        for ki in range(len(weights)):
                            nc.tensor.matmul(
                                pps[0:ms, 0:nn],
                                weights[ki][:, m0 : m0 + ms],
                                xT_sb[ki][:, n0 : n0 + nn],
                                start=(ki == 0),
                                stop=False,
                            )
                        nc.tensor.matmul(
                            pps[0:ms, 0:nn],
                            bias_row[:, m0 : m0 + ms],
                            ones1[:, n0 : n0 + nn],
                            start=False,
                            stop=True,
                        )
                        nc.vector.tensor_copy(
                            dst_tiles[mi][:, n0 : n0 + nn], pps[0:ms, 0:nn]
                        )

            zx_sb = [
                wp.tile([fs, TB], F32, tag=f"zx{fi}", name=f"zx{fi}") for fi, (f0, fs) in enumerate(f_t)
            ]
            precompute(AxT_sb, f_t, zx_sb, attnb_sb)
            cx_sb = [
                wp.tile([ms, TB], F32, tag=f"cx{mi}", name=f"cx{mi}") for mi, (m0, ms) in enumerate(wv_t)
            ]
            precompute(WxT_sb, wv_t, cx_sb, combb_sb)

            if stage < 2:
                return nc
            # ---------------- recurrence ----------------
            c_prev = None
            h_prev = None
            for t in range(T):
                tc0, tc1 = t * Bc, (t + 1) * Bc

                # attention z, h-part  (PE early)
                if t > 0:
                    zh_ps = ps1.tile([128, NF * Bc], F32, tag="sm", name="sm")
                    for mi, (m0, ms) in enumerate(f_t):
                        for ki in range(NH):
                            nc.tensor.matmul(
                                zh_ps[0:ms, mi * Bc : (mi + 1) * Bc],
                                AhT_sb[ki][:, m0 : m0 + ms],
                                h_prev[ki],
                                start=(ki == 0),
                                stop=(ki == NH - 1),
                            )

                # gamma pre-activation (independent of attention; fills PE)
                gam_ps = ps1.tile([Bc, C], F32, tag="gam", name="gam")
                if t > 0:
                    for ki in range(NH):
                        nc.tensor.matmul(
                            gam_ps[:], h_prev[ki], gateT_sb[ki][:],
                            start=(ki == 0), stop=False,
                        )
                    nc.tensor.matmul(gam_ps[:], ones1[:, 0:Bc], gateB_sb[:],
                                     start=False, stop=True)
                else:
                    nc.tensor.matmul(gam_ps[:], ones1[:, 0:Bc], gateB_sb[:],
                                     start=True, stop=True)

                # expz = exp(zx + zh + attn_b)   [f-tile, Bc] bf16
                expz = []
                for fi, (f0, fs) in enumerate(f_t):
                    ez = ap.tile([fs, Bc], BF16, tag=f"expz{fi}", name=f"expz{fi}")
                    if t > 0:
                        zs = ap.tile([fs, Bc], F32, tag=f"zsum{fi}", name=f"zsum{fi}")
                        nc.vector.tensor_tensor(
                            zs[:], zh_ps[0:fs, fi * Bc : (fi + 1) * Bc],
                            zx_sb[fi][:, tc0:tc1], op=OP.add,
                        )
                        src = zs
                    else:
                        src = zx_sb[fi][:, tc0:tc1]
                    nc.scalar.activation(ez[:], src[:], AF.Exp)
                    expz.append(ez)

                # sum over F (partitions) via ones-matmul -> [Bc, 1]
                se_ps = ps1.tile([Bc, 1], F32, tag="se", name="se")
                for fi in range(NF):
                    nc.tensor.matmul(
                        se_ps[:], expz[fi][:], ones_f[fi][:],
                        start=(fi == 0), stop=(fi == NF - 1),
                    )
                recip = ap.tile([Bc, 1], F32, tag="recip", name="recip")
                nc.vector.reciprocal(recip[:], se_ps[:])

                # einsum: ctx[b,:] = sum_f expz[b,f] * encT[b,f,:]
                # 3 concurrent column strips (psum rows 0/32/64); strip j owns
                # samples [off_j, off_j + sz_j); round r of strip j -> sample
                # off_j + r, staged in blk[32j, r, :], gathered per strip by
                # one contiguous SWDGE DMA.
                q3, rem3 = divmod(Bc, 3)
                sizes = [q3 + (1 if j < rem3 else 0) for j in range(3)]
                offs = [0, sizes[0], sizes[0] + sizes[1]]
                NR = sizes[0]
                blk = ap.tile([128, NR, C], BF16, tag="ctxblk", name="ctxblk")
                ctx_sb = ap.tile([Bc, C], BF16, tag="ctx", name="ctx")
                for r in range(NR):
                    strips = [j for j in range(3) if r < sizes[j]]
                    eps = ps2.tile([128, C], F32, tag="ein", name="ein")
                    for s in strips:
                        for fi in range(NF):
                            b = offs[s] + r
                            fs = f_t[fi][1]
                            nc.tensor.matmul(
                                eps[32 * s : 32 * s + 32, :],
                                expz[fi][:, b : b + 1].broadcast_to([fs, 32]),
                                enc_sb[fi][:, b, :],
                                start=(fi == 0),
                                stop=(fi == NF - 1),
                                tile_position=(0, 32 * s),
                            )
                    ge = 32 * strips[-1] + 32
                    nc.scalar.activation(blk[0:ge, r, :], eps[0:ge, :], AF.Copy)
                for j in range(3):
                    if sizes[j] == 0:
                        continue
                    nc.gpsimd.dma_start(
                        ctx_sb[offs[j] : offs[j] + sizes[j], :],
                        blk[32 * j : 32 * j + 1, 0 : sizes[j], :],
                    )

                # fused gate+scale: ctxg = (ctx * 1/Z) * sigmoid(gamma)
                gam_sb = ap.tile([Bc, C], F32, tag="gam_sb", name="gam_sb")
                nc.scalar.activation(gam_sb[:], gam_ps[:], AF.Sigmoid)
                ctxg = ap.tile([Bc, C], F32, tag="ctxg", name="ctxg")
                nc.vector.scalar_tensor_tensor(
                    ctxg[:], ctx_sb[:], recip[:], gam_sb[:],
                    op0=OP.mult, op1=OP.mult,
                )

                # transpose ctxg to fb layout [C-tiles, Bc], cast bf16
                ctT_ps = ps1.tile([128, NH, Bc], F32, tag="sm", name="sm")
                for j in range(NH):
                    nc.tensor.transpose(
                        ctT_ps[:, j, :], ctxg[:, j * 128 : (j + 1) * 128], eye_sb[:]
                    )
                ctxgT = ap.tile([128, NH, Bc], BF16, tag="ctxgT", name="ctxgT")
                nc.vector.tensor_copy(ctxgT[:], ctT_ps[:])

                # comb: inp = relu(cx + WcT.T@ctxgT + comb_b) -> bf16 fb
                cb_ps = ps1.tile([128, NWV, Bc], F32, tag="sm", name="sm")
                for mi, (m0, ms) in enumerate(wv_t):
                    for ki in range(NH):
                        nc.tensor.matmul(
                            cb_ps[0:ms, mi, :],
                            WcT_sb[ki][:, m0 : m0 + ms],
                            ctxgT[:, ki, :],
                            start=(ki == 0),
                            stop=(ki == NH - 1),
                        )
                csum = ap.tile([128, NWV, Bc], F32, tag="csum", name="csum")
                inp_bf = ap.tile([128, NWV, Bc], BF16, tag="inp", name="inp")
                for mi, (m0, ms) in enumerate(wv_t):
                    nc.vector.tensor_tensor(
                        csum[0:ms, mi, :], cb_ps[0:ms, mi, :],
                        cx_sb[mi][:, tc0:tc1], op=OP.add,
                    )
                    nc.scalar.activation(
                        inp_bf[0:ms, mi, :], csum[0:ms, mi, :], AF.Relu
                    )

                # LSTM gates: [128, 16, Bc] psum
                rhs_list = [
                    (inp_bf[0:ks, ki, :], WihT_sb[ki]) for ki, (k0, ks) in enumerate(wv_t)
                ]
                if t > 0:
                    rhs_list += [(h_prev[ki], WhhT_sb[ki]) for ki in range(NH)]
                g_ps = ps1.tile([128, 16, Bc], F32, tag="gates", name="gates")
                for m in range(16):
                    for j, (rhs, wt) in enumerate(rhs_list):
                        nc.tensor.matmul(
                            g_ps[:, m, :],
                            wt[:, m * 128 : (m + 1) * 128],
                            rhs,
                            start=(j == 0),
                            stop=(j == len(rhs_list) - 1),
                        )
                gsum = ap.tile([128, 16, Bc], F32, tag="gsum", name="gsum")
                nc.vector.tensor_tensor(gsum[:], g_ps[:], lstmb_sb[:], op=OP.add)
                nl = ap.tile([128, 16, Bc], F32, tag="nl", name="nl")
                for (a, b_, fn) in (
                    (0, 4, AF.Sigmoid), (4, 8, AF.Sigmoid),
                    (8, 12, AF.Tanh), (12, 16, AF.Sigmoid),
                ):
                    nc.scalar.activation(nl[:, a:b_, :], gsum[:, a:b_, :], fn)
                ig = ap.tile([128, 4, Bc], F32, tag="ig", name="ig")
                nc.vector.tensor_tensor(ig[:], nl[:, 0:4, :], nl[:, 8:12, :], op=OP.mult)
                c_new = st.tile([128, 4, Bc], F32, tag="c", name="c")
                if t > 0:
                    cf = ap.tile([128, 4, Bc], F32, tag="cf", name="cf")
                    nc.vector.tensor_tensor(cf[:], nl[:, 4:8, :], c_prev[:], op=OP.mult)
                    nc.vector.tensor_tensor(c_new[:], ig[:], cf[:], op=OP.add)
                else:
                    nc.vector.tensor_copy(c_new[:], ig[:])
                tanh_c = ap.tile([128, 4, Bc], F32, tag="tanh_c", name="tanh_c")
                nc.scalar.activation(tanh_c[:], c_new[:], AF.Tanh)
                nc.vector.tensor_tensor(
                    h_all[:, :, tc0:tc1], nl[:, 12:16, :], tanh_c[:], op=OP.mult
                )
                c_prev = c_new
                h_prev = [h_all[:, k, tc0:tc1] for k in range(NH)]

            if stage < 3:
                return nc
            # ---------------- vocab projection ----------------
            for n0 in range(0, V, vchunk):
                nn = min(vchunk, V - n0)
                vw = vp.tile([128, NH, vchunk], BF16, tag="vw", name="vw")
                for ki in range(NH):
                    nc.sync.dma_start(
                        vw[:, ki, 0:nn], vWT_d[ki * 128 : (ki + 1) * 128, n0 : n0 + nn]
                    )
                for m0, ms in n_mv:
                    vps = ps2.tile([128, C], F32, tag="ein", name="ein")
                    for ki in range(NH):
                        nc.tensor.matmul(
                            vps[0:ms, 0:nn],
                            h_all[:, ki, m0 : m0 + ms],
                            vw[:, ki, 0:nn],
                            start=(ki == 0),
                            stop=(ki == NH - 1),
                        )
                    vo = vp.tile([128, vchunk], F32, tag="vout", name="vout")
                    nc.scalar.activation(vo[0:ms, 0:nn], vps[0:ms, 0:nn], AF.Copy)
                    nc.sync.dma_start(out_d[m0 : m0 + ms, n0 : n0 + nn], vo[0:ms, 0:nn])

    _split_multi_waits(nc)
    return nc


def _split_multi_waits(nc):
    """walrus' codegen accepts at most one sync wait per engine instruction
    in this environment; hoist extra waits onto same-engine NoOps placed
    immediately before the owning instruction."""
    for fn in nc.m.functions:
        for bb in fn.blocks:
            insts = bb.instructions
            out = []
            changed = False
            for inst in insts:
                si = inst.sync_info
                if si is not None and len(si.on_wait) > 1:
                    waits = list(si.on_wait)
                    for w in waits[:-1]:
                        out.append(
                            mybir.InstNoOp(
                                name=f"{inst.name}-w{len(out)}",
                                engine=inst.engine,
                                sync_info=mybir.SyncInfo(
                                    on_wait=[w], on_update=[]
                                ),
                            )
                        )
                    inst.sync_info = mybir.SyncInfo(
                        on_wait=[waits[-1]], on_update=list(si.on_update)
                    )
                    changed = True
                out.append(inst)
            if changed:
                bb.instructions = out


# ======================= host side =======================

def _bf16(x):
    return np.ascontiguousarray(np.asarray(x, dtype=ml_dtypes.bfloat16))


def _f32(x):
    return np.ascontiguousarray(np.asarray(x, dtype=np.float32))


def prep_shared(inputs, Bc, T, V):
    """Weight-derived in_map entries (replicated across cores)."""
    attn_W = np.asarray(inputs["attn_W"], np.float32)
    comb_W = np.asarray(inputs["comb_W"], np.float32)
    gate_W = np.asarray(inputs["gate_W"], np.float32)
    sh = {
        "AxT": _bf16(attn_W[:, :WV].T),
        "AhT": _bf16(attn_W[:, WV:].T),
        "WxT": _bf16(comb_W[:, :WV].T),
        "WcT": _bf16(comb_W[:, WV:].T),
        "gateTa": _bf16(
            np.concatenate(
                [gate_W.T, np.asarray(inputs["gate_b"], np.float32)[None, :]], 0
            )
        ),
        "WihT": _bf16(np.asarray(inputs["lstm_Wih"]).T),
        "WhhT": _bf16(np.asarray(inputs["lstm_Whh"]).T),
        "vWT": _bf16(np.asarray(inputs["vocab_W"]).T[:, :V]),
        "attn_br": _bf16(np.asarray(inputs["attn_b"])[None, :]),
        "comb_br": _bf16(np.asarray(inputs["comb_b"])[None, :]),
        "eye": np.eye(Bc, dtype=np.float32),
        "ones_col": np.ones((F, 1), dtype=ml_dtypes.bfloat16),
        "ones_row": np.ones((1, T * Bc), dtype=ml_dtypes.bfloat16),
    }
    bsum = (
        np.asarray(inputs["lstm_bih"], np.float32)
        + np.asarray(inputs["lstm_bhh"], np.float32)
    )
    bb = np.ascontiguousarray(bsum.reshape(16, 128).T)  # [128, 16]
    sh["lstm_bc"] = np.ascontiguousarray(
        np.broadcast_to(bb[:, :, None], (128, 16, Bc))
    ).astype(np.float32)
    return sh


def prep_core(inputs, core, Bc, T, V):
    """Batch-sharded in_map entries for one core."""
    b0, b1 = core * Bc, (core + 1) * Bc
    enc = np.asarray(inputs["encoding"], np.float32)[b0:b1]  # [Bc, C, F]
    wv = np.asarray(inputs["wordvecs"], np.float32)[b0:b1, :T]  # [Bc, T, WV]
    x_shift = np.concatenate(
        [np.zeros((Bc, 1, WV), np.float32), wv[:, :-1, :]], axis=1
    )
    return {
        "encT": _bf16(enc.transpose(0, 2, 1)),  # [Bc, F, C]
        "xT": _bf16(x_shift.transpose(2, 1, 0).reshape(WV, T * Bc)),
    }


_PROG_CACHE = {}
LAST_RESULT = None


def kernel(**inputs):
    global LAST_RESULT
    from concourse.bass_utils import run_bass_kernel_spmd

    Bc, T, V = B_FULL // N_CORES, T_FULL, V_FULL
    key = (Bc, T, V)
    if key not in _PROG_CACHE:
        _PROG_CACHE[key] = build_program(Bc, T, V)
    nc = _PROG_CACHE[key]

    shared = prep_shared(inputs, Bc, T, V)
    in_maps = [dict(shared, **prep_core(inputs, k, Bc, T, V)) for k in range(N_CORES)]
    res = run_bass_kernel_spmd(nc, in_maps, list(range(N_CORES)))
    LAST_RESULT = res

    parts = []
    for r in res.results:
        o = np.asarray(r["out"], np.float32).reshape(T, Bc, V).transpose(1, 0, 2)
        parts.append(o)
    out = np.concatenate(parts, axis=0)
    out = out + np.asarray(inputs["vocab_b"], np.float32)[None, None, :]
    return np.ascontiguousarray(out.astype(np.float32))



# revision 12
# speedup vs baseline: 1.1633x; 1.1633x over previous
"""CaptionNet Trainium2 kernel (8-core SPMD, data-parallel over batch).

v2: vocab projection interleaved into the recurrence (keeps PE warm /
HAM at 8/8 and hides the vocab tail), sigmoid-free activations (all ACT
functions from the exp/tanh table set; h and c carried as 2h / 2c with
host-prescaled weights), biases applied via DVE adds against
host-broadcast tiles instead of PE ones-matmuls, softmax 1/Z folded
into a diag(recip) transpose operand, and a host-pretransposed
contiguous enc layout. Vocab weights are split: first VRES n-chunks
resident in SBUF, the rest streamed per 128-token sweep.
"""

import numpy as np
import ml_dtypes

import concourse.bass as bass
import concourse.tile as tile
import concourse.mybir as mybir

BF16 = mybir.dt.bfloat16
F32 = mybir.dt.float32
AF = mybir.ActivationFunctionType
OP = mybir.AluOpType

# Problem constants (full size)
B_FULL, T_FULL, H, WV, F, C, V_FULL = 256, 20, 512, 301, 196, 512, 9871
N_CORES = 8
F_HI = 128
F_LO = F - F_HI  # 68
VCHUNK = 512
VRES = 7          # resident vocab n-chunks (VRES*VCHUNK columns stay in SBUF)
MB = 128          # vocab token-block (4 steps x 32)


def _tiles(total, step=128):
    return [(i, min(step, total - i)) for i in range(0, total, step)]


def build_program(Bc=32, T=20, V=V_FULL, stage=99):
    TB = Bc * T
    NVC = (V + VCHUNK - 1) // VCHUNK  # 20 vocab n-chunks
    nc = bass.Bass()

    # ---------------- DRAM I/O (per-core) ----------------
    # encT: host pre-transposed to [F, Bc, C] so the SBUF load is contiguous
    encT_d = nc.dram_tensor("encT", [F, Bc, C], BF16, kind="ExternalInput")
    xT_d = nc.dram_tensor("xT", [WV, TB], BF16, kind="ExternalInput")
    AxT_d = nc.dram_tensor("AxT", [WV, F], BF16, kind="ExternalInput")
    AhT_d = nc.dram_tensor("AhT", [H, F], BF16, kind="ExternalInput")   # x0.5
    WxT_d = nc.dram_tensor("WxT", [WV, WV], BF16, kind="ExternalInput")
    WcT_d = nc.dram_tensor("WcT", [C, WV], BF16, kind="ExternalInput")
    gateT_d = nc.dram_tensor("gateT", [H, C], BF16, kind="ExternalInput")  # x0.5
    gateB_d = nc.dram_tensor("gateB_bc", [Bc, C], F32, kind="ExternalInput")
    WihT_d = nc.dram_tensor("WihT", [WV, 4 * H], BF16, kind="ExternalInput")
    WhhT_d = nc.dram_tensor("WhhT", [H, 4 * H], BF16, kind="ExternalInput")  # x0.5
    vWT_d = nc.dram_tensor("vWT", [H, V], BF16, kind="ExternalInput")  # x0.5
    attnb_d = nc.dram_tensor("attn_br", [1, F], BF16, kind="ExternalInput")
    combb_d = nc.dram_tensor("comb_br", [1, WV], BF16, kind="ExternalInput")
    lstmb_d = nc.dram_tensor("lstm_bc", [128, 16, Bc], F32, kind="ExternalInput")
    eye_d = nc.dram_tensor("eye", [Bc, Bc], BF16, kind="ExternalInput")
    ones2_d = nc.dram_tensor("ones2_col", [F, 1], BF16, kind="ExternalInput")  # 2.0
    onesr_d = nc.dram_tensor("ones_row", [1, TB], BF16, kind="ExternalInput")
    out_d = nc.dram_tensor("out", [TB, V], F32, kind="ExternalOutput")

    wv_t = _tiles(WV)   # [(0,128),(128,128),(256,45)]
    h_t = _tiles(H)     # 4 x 128
    f_t = [(0, F_HI), (F_HI, F_LO)]
    NWV, NH, NF = len(wv_t), len(h_t), len(f_t)

    with tile.TileContext(nc) as tc:
        with (
            tc.tile_pool(name="w", bufs=1) as wp,
            tc.tile_pool(name="act", bufs=2) as ap,
            tc.tile_pool(name="blk1", bufs=1) as bp,
            tc.tile_pool(name="st", bufs=2) as st,
            tc.tile_pool(name="vw", bufs=2) as vwp,
            tc.tile_pool(name="vo", bufs=2) as vp,
            tc.tile_pool(name="ps_sm", bufs=1, space="PSUM") as ps_sm,
            tc.tile_pool(name="ps_gam", bufs=1, space="PSUM") as ps_gam,
            tc.tile_pool(name="ps_g", bufs=1, space="PSUM") as ps_g,
            tc.tile_pool(name="ps_g2", bufs=1, space="PSUM") as ps_g2,
            tc.tile_pool(name="ps_e", bufs=2, space="PSUM") as ps_e,
            tc.tile_pool(name="ps_v", bufs=2, space="PSUM") as ps_v,
        ):
            # ---------------- resident loads ----------------
            def load_ktiles(pool, dram, ktiles, ncols, dt, name):
                out = []
                for ki, (k0, ks) in enumerate(ktiles):
                    tl = pool.tile([ks, ncols], dt, tag=f"{name}{ki}", name=f"{name}{ki}")
                    nc.sync.dma_start(tl[:], dram[k0 : k0 + ks, :])
                    out.append(tl)
                return out

            xT_sb = load_ktiles(wp, xT_d, wv_t, TB, BF16, "xT")
            AxT_sb = load_ktiles(wp, AxT_d, wv_t, F, BF16, "AxT")
            WxT_sb = load_ktiles(wp, WxT_d, wv_t, WV, BF16, "WxT")
            attnb_sb = wp.tile([1, F], BF16, tag="attnbr", name="attnbr")
            nc.sync.dma_start(attnb_sb[:], attnb_d[:])
            combb_sb = wp.tile([1, WV], BF16, tag="combbr", name="combbr")
            nc.sync.dma_start(combb_sb[:], combb_d[:])
            ones1 = wp.tile([1, TB], BF16, tag="onesr", name="onesr")
            nc.sync.dma_start(ones1[:], onesr_d[:])

            enc_sb = []
            for fi, (f0, fs) in enumerate(f_t):
                e = wp.tile([fs, Bc, C], BF16, tag=f"enc{fi}", name=f"enc{fi}")
                nc.sync.dma_start(e[:], encT_d[f0 : f0 + fs])
                enc_sb.append(e)

            AhT_sb = load_ktiles(wp, AhT_d, h_t, F, BF16, "AhT")
            WcT_sb = load_ktiles(wp, WcT_d, h_t, WV, BF16, "WcT")
            gateT_sb = load_ktiles(wp, gateT_d, h_t, C, BF16, "gateT")
            WihT_sb = load_ktiles(wp, WihT_d, wv_t, 4 * H, BF16, "WihT")
            WhhT_sb = load_ktiles(wp, WhhT_d, h_t, 4 * H, BF16, "WhhT")
            gateB_sb = wp.tile([Bc, C], F32, tag="gateB", name="gateB")
            nc.sync.dma_start(gateB_sb[:], gateB_d[:])
            lstmb_sb = wp.tile([128, 16, Bc], F32, tag="lstmb", name="lstmb")
            nc.sync.dma_start(lstmb_sb[:], lstmb_d[:])
            eye_sb = wp.tile([Bc, Bc], BF16, tag="eye", name="eye")
            nc.sync.dma_start(eye_sb[:], eye_d[:])
            ones2_f = load_ktiles(wp, ones2_d, f_t, 1, BF16, "ones2")

            # resident slice of vocab weights: n-chunks [0, VRES)
            vres_cols = min(VRES * VCHUNK, V)
            vwA = wp.tile([128, NH, vres_cols], BF16, tag="vwA", name="vwA")
            for ki in range(NH):
                nc.sync.dma_start(
                    vwA[:, ki, :], vWT_d[ki * 128 : (ki + 1) * 128, 0:vres_cols]
                )

            h_all = wp.tile([128, NH, TB], BF16, tag="h_all", name="h_all")

            if stage < 1:
                return nc
            # ---------------- x-precomputes: zx = x@Ax.T+ab ; cx = x@Wx.T+cb ----------------
            NN = min(320, TB)

            zx_sb = [
                wp.tile([fs, TB], BF16, tag=f"zx{fi}", name=f"zx{fi}")
                for fi, (f0, fs) in enumerate(f_t)
            ]
            cx_sb = wp.tile([128, NWV, TB], BF16, tag="cx", name="cx")

            def precompute(weights, mtiles, dst, bias_row):
                for mi, (m0, ms) in enumerate(mtiles):
                    for n0 in range(0, TB, NN):
                        nn_ = min(NN, TB - n0)
                        pps = ps_sm.tile([128, NN], F32, tag="sm", name="sm")
                        for ki in range(len(weights)):
                            nc.tensor.matmul(
                                pps[0:ms, 0:nn_],
                                weights[ki][:, m0 : m0 + ms],
                                xT_sb[ki][:, n0 : n0 + nn_],
                                start=(ki == 0),
                                stop=False,
                            )
                        nc.tensor.matmul(
                            pps[0:ms, 0:nn_],
                            bias_row[:, m0 : m0 + ms],
                            ones1[:, n0 : n0 + nn_],
                            start=False,
                            stop=True,
                        )
                        dst(mi, m0, ms, n0, nn_, pps)

            precompute(
                AxT_sb, f_t,
                lambda mi, m0, ms, n0, nn_, pps: nc.vector.tensor_copy(
                    zx_sb[mi][:, n0 : n0 + nn_], pps[0:ms, 0:nn_]
                ),
                attnb_sb,
            )
            precompute(
                WxT_sb, wv_t,
                lambda mi, m0, ms, n0, nn_, pps: nc.vector.tensor_copy(
                    cx_sb[0:ms, mi, n0 : n0 + nn_], pps[0:ms, 0:nn_]
                ),
                combb_sb,
            )

            if stage < 2:
                return nc

            # ---------------- vocab sweep emission ----------------
            def emit_vocab_chunk(b, ci):
                m0 = b * MB
                n0 = ci * VCHUNK
                nn_ = min(VCHUNK, V - n0)
                if n0 < vres_cols:
                    vw_view = [vwA[:, ki, n0 : n0 + nn_] for ki in range(NH)]
                else:
                    vw = vwp.tile([128, NH, VCHUNK], BF16, tag="vw", name="vw")
                    for ki in range(NH):
                        nc.sync.dma_start(
                            vw[:, ki, 0:nn_],
                            vWT_d[ki * 128 : (ki + 1) * 128, n0 : n0 + nn_],
                        )
                    vw_view = [vw[:, ki, 0:nn_] for ki in range(NH)]
                vps = ps_v.tile([128, VCHUNK], F32, tag="vps", name="vps")
                for ki in range(NH):
                    nc.tensor.matmul(
                        vps[:, 0:nn_],
                        h_all[:, ki, m0 : m0 + MB],
                        vw_view[ki],
                        start=(ki == 0),
                        stop=(ki == NH - 1),
                    )
                vo = vp.tile([128, VCHUNK], F32, tag="vout", name="vout")
                nc.vector.tensor_copy(vo[:, 0:nn_], vps[:, 0:nn_])
                nc.sync.dma_start(out_d[m0 : m0 + MB, n0 : n0 + nn_], vo[:, 0:nn_])

            def sweep_chunks(t):
                # block b = (t-4)//4 swept over steps 4b+4 .. 4b+7, 5 chunks/step
                if t < 4:
                    return []
                b = (t - 4) // 4
                c0 = ((t - 4) % 4) * 5
                return [(b, c) for c in range(c0, min(c0 + 5, NVC))]

            # ---------------- recurrence ----------------
            C2_prev = None
            h_prev = None
            for t in range(T):
                tc0, tc1 = t * Bc, (t + 1) * Bc
                chunks = sweep_chunks(t)

                # ---- early PE work (depends only on h_prev) ----
                if t > 0:
                    zh_ps = ps_sm.tile([128, NF * Bc], F32, tag="sm", name="sm")
                    for mi, (m0, ms) in enumerate(f_t):
                        for ki in range(NH):
                            nc.tensor.matmul(
                                zh_ps[0:ms, mi * Bc : (mi + 1) * Bc],
                                AhT_sb[ki][:, m0 : m0 + ms],
                                h_prev[ki],
                                start=(ki == 0),
                                stop=(ki == NH - 1),
                            )
                    gam_ps = ps_gam.tile([Bc, C], F32, tag="gam", name="gam")
                    for ki in range(NH):
                        nc.tensor.matmul(
                            gam_ps[:], h_prev[ki], gateT_sb[ki][:],
                            start=(ki == 0), stop=(ki == NH - 1),
                        )

                # LSTM h-part: fills PE while attention path round-trips
                # (own PSUM tile; closed accumulation groups per m-plane)
                if t > 0:
                    g_ps = ps_g.tile([128, 16, Bc], F32, tag="gates", name="gates")
                    for m in range(16):
                        for ki in range(NH):
                            nc.tensor.matmul(
                                g_ps[:, m, :],
                                WhhT_sb[ki][:, m * 128 : (m + 1) * 128],
                                h_prev[ki],
                                start=(ki == 0),
                                stop=(ki == NH - 1),
                            )

                # ---- expz = exp(zx + zh) ----
                expz = []
                for fi, (f0, fs) in enumerate(f_t):
                    ez = ap.tile([fs, Bc], BF16, tag=f"expz{fi}", name=f"expz{fi}")
                    if t > 0:
                        zs = ap.tile([fs, Bc], F32, tag=f"zsum{fi}", name=f"zsum{fi}")
                        nc.vector.tensor_tensor(
                            zs[:], zh_ps[0:fs, fi * Bc : (fi + 1) * Bc],
                            zx_sb[fi][:, tc0:tc1], op=OP.add,
                        )
                        src = zs[:]
                    else:
                        src = zx_sb[fi][:, tc0:tc1]
                    nc.scalar.activation(ez[:], src, AF.Exp)
                    expz.append(ez)

                # se = 2*sum(exp) -> recip = 1/(2Z)   (0.5 folded for gating)
                se_ps = ps_sm.tile([Bc, 1], F32, tag="sm", name="sm")
                for fi in range(NF):
                    nc.tensor.matmul(
                        se_ps[0:Bc, 0:1], expz[fi][:], ones2_f[fi][:],
                        start=(fi == 0), stop=(fi == NF - 1),
                    )
                recip = ap.tile([Bc, 1], F32, tag="recip", name="recip")
                nc.vector.reciprocal(recip[:], se_ps[0:Bc, 0:1])
                # diagR = eye * recip  (per-sample 1/(2Z) on the diagonal)
                diagR = ap.tile([Bc, Bc], BF16, tag="diagR", name="diagR")
                nc.vector.tensor_scalar(
                    diagR[:], eye_sb[:], recip[:], None, op0=OP.mult
                )

                # gamma: tgam = tanh(0.5*(gam + gate_b))
                tgam = ap.tile([Bc, C], BF16, tag="tgam", name="tgam")
                if t > 0:
                    gsum = ap.tile([Bc, C], F32, tag="gsum", name="gsum")
                    nc.vector.tensor_tensor(gsum[:], gam_ps[:], gateB_sb[:], op=OP.add)
                    nc.scalar.activation(tgam[:], gsum[:], AF.Tanh, scale=0.5)
                else:
                    nc.scalar.activation(tgam[:], gateB_sb[:], AF.Tanh, scale=0.5)

                # ---- einsum: ctx[b,:] = sum_f expz[b,f] * encT[b,f,:] ----
                q3, rem3 = divmod(Bc, 3)
                sizes = [q3 + (1 if j < rem3 else 0) for j in range(3)]
                offs = [0, sizes[0], sizes[0] + sizes[1]]
                NR = sizes[0]
                blk = bp.tile([128, NR, C], BF16, tag="ctxblk", name="ctxblk")
                ctx_sb = ap.tile([Bc, C], BF16, tag="ctx", name="ctx")
                for r in range(NR):
                    strips = [j for j in range(3) if r < sizes[j]]
                    eps = ps_e.tile([128, C], F32, tag="ein", name="ein")
                    for s in strips:
                        for fi in range(NF):
                            b = offs[s] + r
                            fs = f_t[fi][1]
                            nc.tensor.matmul(
                                eps[32 * s : 32 * s + 32, :],
                                expz[fi][:, b : b + 1].broadcast_to([fs, 32]),
                                enc_sb[fi][:, b, :],
                                start=(fi == 0),
                                stop=(fi == NF - 1),
                                tile_position=(0, 32 * s),
                            )
                    ge = 32 * strips[-1] + 32
                    if r % 2 == 0:
                        nc.scalar.activation(blk[0:ge, r, :], eps[0:ge, :], AF.Copy)
                    else:
                        nc.vector.tensor_copy(blk[0:ge, r, :], eps[0:ge, :])

                # vocab interleave A (PE hole while gather/gate path runs)
                for (b, ci) in chunks[:2]:
                    emit_vocab_chunk(b, ci)

                for j in range(3):
                    if sizes[j] == 0:
                        continue
                    nc.gpsimd.dma_start(
                        ctx_sb[offs[j] : offs[j] + sizes[j], :],
                        blk[32 * j : 32 * j + 1, 0 : sizes[j], :],
                    )

                # u = (tgam + 1) * ctx   [Bc, C] bf16
                u = ap.tile([Bc, C], BF16, tag="u", name="u")
                nc.vector.scalar_tensor_tensor(
                    u[:], tgam[:], 1.0, ctx_sb[:], op0=OP.add, op1=OP.mult
                )

                # transpose u (applying diagR): ctT[c,b] = sum_b' u[b',c]*diagR[b',b]
                ctT_ps = ps_sm.tile([128, NH, Bc], F32, tag="sm", name="sm")
                for j in range(NH):
                    nc.tensor.matmul(
                        ctT_ps[:, j, :], u[:, j * 128 : (j + 1) * 128], diagR[:],
                        start=True, stop=True,
                    )
                ctxgT = ap.tile([128, NH, Bc], BF16, tag="ctxgT", name="ctxgT")
                nc.vector.tensor_copy(ctxgT[:], ctT_ps[:])

                # comb: inp = relu(cx + WcT.T@ctxgT) -> bf16 fb
                cb_ps = ps_sm.tile([128, NWV, Bc], F32, tag="sm", name="sm")
                for mi, (m0, ms) in enumerate(wv_t):
                    for ki in range(NH):
                        nc.tensor.matmul(
                            cb_ps[0:ms, mi, :],
                            WcT_sb[ki][:, m0 : m0 + ms],
                            ctxgT[:, ki, :],
                            start=(ki == 0),
                            stop=(ki == NH - 1),
                        )
                csum = ap.tile([128, NWV, Bc], F32, tag="csum", name="csum")
                inp_bf = ap.tile([128, NWV, Bc], BF16, tag="inp", name="inp")
                nc.vector.tensor_tensor(
                    csum[:, 0:2, :], cb_ps[:, 0:2, :],
                    cx_sb[:, 0:2, tc0:tc1], op=OP.add,
                )
                ks2 = wv_t[2][1]
                nc.vector.tensor_tensor(
                    csum[0:ks2, 2, :], cb_ps[0:ks2, 2, :],
                    cx_sb[0:ks2, 2, tc0:tc1], op=OP.add,
                )
                nc.vector.tensor_scalar(
                    inp_bf[:, 0:2, :], csum[:, 0:2, :], 0.0, None, op0=OP.max
                )
                nc.vector.tensor_scalar(
                    inp_bf[0:ks2, 2, :], csum[0:ks2, 2, :], 0.0, None, op0=OP.max
                )

                # LSTM inp-part (second PSUM tile; summed with h-part below)
                g2_ps = ps_g2.tile([128, 16, Bc], F32, tag="gates2", name="gates2")
                for m in range(16):
                    for ki, (k0, ks) in enumerate(wv_t):
                        nc.tensor.matmul(
                            g2_ps[:, m, :],
                            WihT_sb[ki][:, m * 128 : (m + 1) * 128],
                            inp_bf[0:ks, ki, :],
                            start=(ki == 0),
                            stop=(ki == NWV - 1),
                        )

                # vocab interleave B (PE tail while LSTM elementwise runs)
                for (b, ci) in chunks[2:]:
                    emit_vocab_chunk(b, ci)

                # ---- LSTM elementwise (tanh-only; state carried as 2c / 2h) ----
                gsum2 = ap.tile([128, 16, Bc], F32, tag="gsum2", name="gsum2")
                if t > 0:
                    gtmp = ap.tile([128, 16, Bc], F32, tag="gtmp", name="gtmp")
                    nc.vector.tensor_tensor(gtmp[:], g2_ps[:], lstmb_sb[:], op=OP.add)
                    nc.vector.tensor_tensor(gsum2[:], gtmp[:], g_ps[:], op=OP.add)
                else:
                    nc.vector.tensor_tensor(gsum2[:], g2_ps[:], lstmb_sb[:], op=OP.add)
                nl = ap.tile([128, 16, Bc], F32, tag="nl", name="nl")
                nc.scalar.activation(nl[:, 0:8, :], gsum2[:, 0:8, :], AF.Tanh, scale=0.5)
                nc.scalar.activation(nl[:, 8:12, :], gsum2[:, 8:12, :], AF.Tanh)
                nc.scalar.activation(nl[:, 12:16, :], gsum2[:, 12:16, :], AF.Tanh, scale=0.5)
                # B = (ti+1)*tg ; P = (tf+1)*C2_prev ; C2 = 0.5*P + B
                Bt = ap.tile([128, 4, Bc], F32, tag="Bt", name="Bt")
                nc.vector.scalar_tensor_tensor(
                    Bt[:], nl[:, 0:4, :], 1.0, nl[:, 8:12, :], op0=OP.add, op1=OP.mult
                )
                C2 = st.tile([128, 4, Bc], F32, tag="c2", name="c2")
                if t > 0:
                    Pt = ap.tile([128, 4, Bc], F32, tag="Pt", name="Pt")
                    nc.vector.scalar_tensor_tensor(
                        Pt[:], nl[:, 4:8, :], 1.0, C2_prev[:], op0=OP.add, op1=OP.mult
                    )
                    nc.vector.scalar_tensor_tensor(
                        C2[:], Pt[:], 0.5, Bt[:], op0=OP.mult, op1=OP.add
                    )
                else:
                    nc.vector.tensor_copy(C2[:], Bt[:])
                tanh_c = ap.tile([128, 4, Bc], F32, tag="tanh_c", name="tanh_c")
                nc.scalar.activation(tanh_c[:], C2[:], AF.Tanh, scale=0.5)
                # H2 = (to+1)*tanh(c) = 2h  -> h_all (weights pre-halved)
                nc.vector.scalar_tensor_tensor(
                    h_all[:, :, tc0:tc1], nl[:, 12:16, :], 1.0, tanh_c[:],
                    op0=OP.add, op1=OP.mult,
                )
                C2_prev = C2
                h_prev = [h_all[:, k, tc0:tc1] for k in range(NH)]

            if stage < 3:
                return nc
            # ---------------- final vocab sweep (block 4) ----------------
            for ci in range(NVC):
                emit_vocab_chunk(T * Bc // MB - 1, ci)

    _split_multi_waits(nc)
    return nc


def _split_multi_waits(nc):
    """walrus' codegen accepts at most one sync wait per engine instruction
    in this environment; hoist extra waits onto same-engine NoOps placed
    immediately before the owning instruction."""
    for fn in nc.m.functions:
        for bb in fn.blocks:
            insts = bb.instructions
            out = []
            changed = False
            for inst in insts:
                si = inst.sync_info
                if si is not None and len(si.on_wait) > 1:
                    waits = list(si.on_wait)
                    for w in waits[:-1]:
                        out.append(
                            mybir.InstNoOp(
                                name=f"{inst.name}-w{len(out)}",
                                engine=inst.engine,
                                sync_info=mybir.SyncInfo(
                                    on_wait=[w], on_update=[]
                                ),
                            )
                        )
                    inst.sync_info = mybir.SyncInfo(
                        on_wait=[waits[-1]], on_update=list(si.on_update)
                    )
                    changed = True
                out.append(inst)
            if changed:
                bb.instructions = out


# ======================= host side =======================

def _bf16(x):
    return np.ascontiguousarray(np.asarray(x, dtype=ml_dtypes.bfloat16))


def prep_shared(inputs, Bc, T, V):
    """Weight-derived in_map entries (replicated across cores)."""
    attn_W = np.asarray(inputs["attn_W"], np.float32)
    comb_W = np.asarray(inputs["comb_W"], np.float32)
    gate_W = np.asarray(inputs["gate_W"], np.float32)
    gate_b = np.asarray(inputs["gate_b"], np.float32)
    sh = {
        "AxT": _bf16(attn_W[:, :WV].T),
        "AhT": _bf16(attn_W[:, WV:].T * 0.5),
        "WxT": _bf16(comb_W[:, :WV].T),
        "WcT": _bf16(comb_W[:, WV:].T),
        "gateT": _bf16(gate_W.T * 0.5),
        "gateB_bc": np.ascontiguousarray(
            np.broadcast_to(gate_b[None, :], (Bc, C)).astype(np.float32)
        ),
        "WihT": _bf16(np.asarray(inputs["lstm_Wih"]).T),
        "WhhT": _bf16(np.asarray(inputs["lstm_Whh"]).T * 0.5),
        "vWT": _bf16(np.asarray(inputs["vocab_W"]).T[:, :V] * 0.5),
        "attn_br": _bf16(np.asarray(inputs["attn_b"])[None, :]),
        "comb_br": _bf16(np.asarray(inputs["comb_b"])[None, :]),
        "eye": np.eye(Bc, dtype=ml_dtypes.bfloat16),
        "ones2_col": np.full((F, 1), 2.0, dtype=ml_dtypes.bfloat16),
        "ones_row": np.ones((1, T * Bc), dtype=ml_dtypes.bfloat16),
    }
    bsum = (
        np.asarray(inputs["lstm_bih"], np.float32)
        + np.asarray(inputs["lstm_bhh"], np.float32)
    )
    bb = np.ascontiguousarray(bsum.reshape(16, 128).T)  # [128, 16]
    sh["lstm_bc"] = np.ascontiguousarray(
        np.broadcast_to(bb[:, :, None], (128, 16, Bc))
    ).astype(np.float32)
    return sh


def prep_core(inputs, core, Bc, T, V):
    """Batch-sharded in_map entries for one core."""
    b0, b1 = core * Bc, (core + 1) * Bc
    enc = np.asarray(inputs["encoding"], np.float32)[b0:b1]  # [Bc, C, F]
    wv = np.asarray(inputs["wordvecs"], np.float32)[b0:b1, :T]  # [Bc, T, WV]
    x_shift = np.concatenate(
        [np.zeros((Bc, 1, WV), np.float32), wv[:, :-1, :]], axis=1
    )
    return {
        # [F, Bc, C]: contiguous per-partition rows for the SBUF load
        "encT": _bf16(enc.transpose(2, 0, 1)),
        "xT": _bf16(x_shift.transpose(2, 1, 0).reshape(WV, T * Bc)),
    }


_PROG_CACHE = {}
LAST_RESULT = None


def kernel(**inputs):
    global LAST_RESULT
    from concourse.bass_utils import run_bass_kernel_spmd

    Bc, T, V = B_FULL // N_CORES, T_FULL, V_FULL
    key = (Bc, T, V)
    if key not in _PROG_CACHE:
        _PROG_CACHE[key] = build_program(Bc, T, V)
    nc = _PROG_CACHE[key]

    shared = prep_shared(inputs, Bc, T, V)
    in_maps = [dict(shared, **prep_core(inputs, k, Bc, T, V)) for k in range(N_CORES)]
    res = run_bass_kernel_spmd(nc, in_maps, list(range(N_CORES)))
    LAST_RESULT = res

    parts = []
    for r in res.results:
        o = np.asarray(r["out"], np.float32).reshape(T, Bc, V).transpose(1, 0, 2)
        parts.append(o)
    out = np.concatenate(parts, axis=0)
    out = out + np.asarray(inputs["vocab_b"], np.float32)[None, None, :]
    return np.ascontiguousarray(out.astype(np.float32))
